# revision 4
# baseline (speedup 1.0000x reference)
"""Trainium2 Bass kernel for nn_NodeNetwork (GNN message passing + MLP + L2 norm).

Data-parallel over nodes: 500000 nodes sharded as 62500/core across 8
NeuronCores; MLP weights replicated.

Per-core layout (v2): global node map node = 500*p + u with p in [0,125)
partitions, u in [0,500). Message is streamed in 125 macro-tiles of 4
u-slots (32 KB contiguous per partition per DMA); features/global/out are
moved in 25 supertiles of 20 u-slots (10 KB contiguous per partition).
This keeps every DMA descriptor >= 10 KB, which is what the HBM DMA path
needs to run near line rate.
"""

import numpy as np

F = 128
DEG = 16
H1 = 256
H2 = 256
OUT = 128
N_CORES = 8

P = 125          # partitions used (node band per partition = n_nodes / P)
UM = 4           # u-slots per macro-tile (nodes per partition per macro)
US = 20          # u-slots per supertile (feat/glob/out granularity)
MACRO = P * UM   # 500 nodes per macro-tile

_NC_CACHE = {}


def build(n_nodes, n_cores=N_CORES, mode="full"):
    import concourse.bacc as bacc
    import concourse.mybir as mybir
    import concourse.tile as tile
    import concourse.masks as masks
    from contextlib import ExitStack

    f32 = mybir.dt.float32
    AX = mybir.AxisListType
    ALU = mybir.AluOpType
    ACTF = mybir.ActivationFunctionType

    assert n_nodes % (P * US) == 0
    U = n_nodes // P             # u-slots per partition (500)
    n_super = U // US            # 25
    mpg = US // UM               # 5 macros per supertile
    DF = DEG * F                 # 2048

    nc = bacc.Bacc(
        "TRN2", target_bir_lowering=False, debug=False, num_devices=n_cores
    )
    msg_d = nc.dram_tensor("message", [n_nodes, DF], f32, kind="ExternalInput").ap()
    feat_d = nc.dram_tensor("features", [n_nodes, F], f32, kind="ExternalInput").ap()
    glob_d = nc.dram_tensor(
        "global_features", [n_nodes, F], f32, kind="ExternalInput"
    ).ap()
    w1_d = nc.dram_tensor("W1", [3 * F, H1], f32, kind="ExternalInput").ap()
    b1_d = nc.dram_tensor("b1", [H1], f32, kind="ExternalInput").ap()
    w2_d = nc.dram_tensor("W2", [H1, H2], f32, kind="ExternalInput").ap()
    b2_d = nc.dram_tensor("b2", [H2], f32, kind="ExternalInput").ap()
    w3_d = nc.dram_tensor("W3", [H2, OUT], f32, kind="ExternalInput").ap()
    b3_d = nc.dram_tensor("b3", [OUT], f32, kind="ExternalInput").ap()
    out_d = nc.dram_tensor("out", [n_nodes, OUT], f32, kind="ExternalOutput").ap()

    msg_r = msg_d.rearrange("(p u) j -> p u j", p=P)
    feat_r = feat_d.rearrange("(p u) f -> p u f", p=P)
    glob_r = glob_d.rearrange("(p u) f -> p u f", p=P)
    out_r = out_d.rearrange("(p u) f -> p u f", p=P)

    with tile.TileContext(nc) as tc, ExitStack() as ctx:
        cpool = ctx.enter_context(tc.tile_pool(name="consts", bufs=1))
        mpool = ctx.enter_context(tc.tile_pool(name="msg", bufs=3))
        ipool = ctx.enter_context(tc.tile_pool(name="inputs", bufs=2))
        xpool = ctx.enter_context(tc.tile_pool(name="xside", bufs=2))
        xtpool = ctx.enter_context(tc.tile_pool(name="xt", bufs=3))
        hpool = ctx.enter_context(tc.tile_pool(name="hid", bufs=2))
        npool = ctx.enter_context(tc.tile_pool(name="norm", bufs=2))
        opool = ctx.enter_context(tc.tile_pool(name="outp", bufs=2))
        ps_xt = ctx.enter_context(tc.tile_pool(name="ps_xt", bufs=2, space="PSUM"))
        ps_mm = ctx.enter_context(tc.tile_pool(name="ps_mm", bufs=3, space="PSUM"))
        ps_out = ctx.enter_context(tc.tile_pool(name="ps_out", bufs=2, space="PSUM"))

        # --- constants ---
        ident = cpool.tile([128, 128], f32, tag="ident")
        masks.make_identity(nc, ident[:])
        w1sb = cpool.tile([128, 3 * H1], f32, tag="w1")  # [p, (k=3, m*128+c=256)]
        nc.sync.dma_start(w1sb[:], w1_d.rearrange("(k p) m -> p k m", p=128))
        w2sb = cpool.tile([128, 2 * H2], f32, tag="w2")
        nc.sync.dma_start(w2sb[:], w2_d.rearrange("(k p) m -> p k m", p=128))
        w3sb = cpool.tile([128, 2 * OUT], f32, tag="w3")
        nc.sync.dma_start(w3sb[:], w3_d.rearrange("(k p) m -> p k m", p=128))
        b1sb = cpool.tile([128, 2], f32, tag="b1")
        nc.sync.dma_start(b1sb[:], b1_d.rearrange("(m p) -> p m", p=128))
        b2sb = cpool.tile([128, 2], f32, tag="b2")
        nc.sync.dma_start(b2sb[:], b2_d.rearrange("(m p) -> p m", p=128))
        b3sb = cpool.tile([128, 1], f32, tag="b3")
        nc.sync.dma_start(b3sb[:], b3_d.rearrange("(m p) -> p m", p=128))

        if mode == "noin":
            z = opool.tile([128, US * F], f32, tag="outsb")
            nc.gpsimd.memset(z[:], 0.25)
            for s in range(n_super):
                nc.scalar.dma_start(
                    out_r[:, s * US : (s + 1) * US],
                    z[:P].rearrange("p (u f) -> p u f", u=US),
                )
            n_super = 0

        for s in range(n_super):
            # --- supertile loads: features/global (10 KB contiguous/partition)
            featt = ipool.tile([128, US * F], f32, tag="featt")
            nc.scalar.dma_start(
                featt[:P].rearrange("p (u f) -> p u f", u=US),
                feat_r[:, s * US : (s + 1) * US],
            )
            globt = ipool.tile([128, US * F], f32, tag="globt")
            nc.scalar.dma_start(
                globt[:P].rearrange("p (u f) -> p u f", u=US),
                glob_r[:, s * US : (s + 1) * US],
            )
            outsb = opool.tile([128, US * F], f32, tag="outsb")

            for q in range(mpg):
                u0 = s * US + q * UM
                # --- message macro-tile load (32 KB contiguous/partition) ---
                msgt = mpool.tile([128, UM * DF], f32, tag="msgt")
                nc.sync.dma_start(
                    msgt[:P].rearrange("p (u j) -> p u j", u=UM),
                    msg_r[:, u0 : u0 + UM],
                )

                if mode == "dma_f":
                    nc.vector.tensor_add(
                        outsb[:P, q * UM * F : (q + 1) * UM * F],
                        msgt[:P, : UM * F],
                        featt[:P, q * UM * F : (q + 1) * UM * F],
                    )
                    continue

                # --- mailbox sum over DEG (node-major) ---
                xagg = xpool.tile([128, UM * F], f32, tag="xagg")
                for g in range(UM):
                    nc.vector.tensor_reduce(
                        xagg[:P, g * F : (g + 1) * F],
                        msgt[:P, g * DF : (g + 1) * DF].rearrange(
                            "p (d f) -> p f d", f=F
                        ),
                        axis=AX.X,
                        op=ALU.add,
                    )

                if mode == "dma_agg":
                    nc.vector.tensor_add(
                        outsb[:P, q * UM * F : (q + 1) * UM * F],
                        xagg[:P],
                        featt[:P, q * UM * F : (q + 1) * UM * F],
                    )
                    continue

                # --- transpose x pieces to feature-major [128, 500] ---
                xts = []
                for src, off in (
                    (xagg, 0),
                    (featt, q * UM * F),
                    (globt, q * UM * F),
                ):
                    pxt = ps_xt.tile([128, 512], f32, tag="pxt")
                    for g in range(UM):
                        nc.tensor.transpose(
                            pxt[:, g * P : (g + 1) * P],
                            src[:P, off + g * F : off + (g + 1) * F],
                            ident[:P, :P],
                        )
                    xt = xtpool.tile([128, MACRO], f32, tag="xt")
                    nc.scalar.copy(xt[:], pxt[:, :MACRO])
                    xts.append(xt)

                # --- layer 1: [384 -> 256], relu ---
                h1 = hpool.tile([128, 2 * MACRO], f32, tag="h1")
                for m in range(2):
                    pmm = ps_mm.tile([128, MACRO], f32, tag="pmm")
                    for k in range(3):
                        nc.tensor.matmul(
                            pmm[:],
                            w1sb[:, k * H1 + m * 128 : k * H1 + (m + 1) * 128],
                            xts[k][:],
                            start=(k == 0),
                            stop=(k == 2),
                        )
                    nc.scalar.activation(
                        h1[:, m * MACRO : (m + 1) * MACRO],
                        pmm[:],
                        ACTF.Relu,
                        bias=b1sb[:, m : m + 1],
                    )

                # --- layer 2: [256 -> 256], relu ---
                h2 = hpool.tile([128, 2 * MACRO], f32, tag="h2")
                for m in range(2):
                    pmm = ps_mm.tile([128, MACRO], f32, tag="pmm")
                    for k in range(2):
                        nc.tensor.matmul(
                            pmm[:],
                            w2sb[:, k * H2 + m * 128 : k * H2 + (m + 1) * 128],
                            h1[:, k * MACRO : (k + 1) * MACRO],
                            start=(k == 0),
                            stop=(k == 1),
                        )
                    nc.scalar.activation(
                        h2[:, m * MACRO : (m + 1) * MACRO],
                        pmm[:],
                        ACTF.Relu,
                        bias=b2sb[:, m : m + 1],
                    )

                # --- layer 3: [256 -> 128], + b3 ---
                pmm = ps_mm.tile([128, MACRO], f32, tag="pmm")
                for k in range(2):
                    nc.tensor.matmul(
                        pmm[:],
                        w3sb[:, k * OUT : (k + 1) * OUT],
                        h2[:, k * MACRO : (k + 1) * MACRO],
                        start=(k == 0),
                        stop=(k == 1),
                    )
                o3 = hpool.tile([128, MACRO], f32, tag="o3")
                nc.scalar.activation(o3[:], pmm[:], ACTF.Identity, bias=b3sb[:, 0:1])

                # --- transpose back to node-major ---
                pout = ps_out.tile([128, UM * F], f32, tag="pout")
                for g in range(UM):
                    nc.tensor.transpose(
                        pout[:P, g * F : (g + 1) * F],
                        o3[:, g * P : (g + 1) * P],
                        ident[:, :],
                    )

                # --- row L2 norm ---
                sq = npool.tile([128, UM * F], f32, tag="sq")
                nsq = npool.tile([128, UM], f32, tag="nsq")
                for g in range(UM):
                    nc.scalar.activation(
                        sq[:P, g * F : (g + 1) * F],
                        pout[:P, g * F : (g + 1) * F],
                        ACTF.Square,
                        accum_out=nsq[:P, g : g + 1],
                    )
                nv = npool.tile([128, UM], f32, tag="nv")
                nc.scalar.activation(nv[:P], nsq[:P], ACTF.Sqrt)
                nve = npool.tile([128, UM], f32, tag="nve")
                nc.vector.tensor_scalar_add(nve[:P], nv[:P], 1e-8)
                ri = npool.tile([128, UM], f32, tag="ri")
                nc.vector.reciprocal(ri[:P], nve[:P])

                for g in range(UM):
                    nc.vector.tensor_scalar_mul(
                        outsb[:P, (q * UM + g) * F : (q * UM + g + 1) * F],
                        pout[:P, g * F : (g + 1) * F],
                        ri[:P, g : g + 1],
                    )

            # --- supertile store (10 KB contiguous/partition) ---
            nc.scalar.dma_start(
                out_r[:, s * US : (s + 1) * US],
                outsb[:P].rearrange("p (u f) -> p u f", u=US),
            )

    nc.compile()
    return nc


def _get_nc(n_nodes, n_cores):
    key = (n_nodes, n_cores)
    if key not in _NC_CACHE:
        _NC_CACHE[key] = build(n_nodes, n_cores)
    return _NC_CACHE[key]


def kernel(message, features, global_features, W1, b1, W2, b2, W3, b3):
    from concourse.bass_utils import run_bass_kernel_spmd

    n = message.shape[0]
    assert n % N_CORES == 0
    npc = n // N_CORES

    nc = _get_nc(npc, N_CORES)

    def shard(a, shape):
        return np.ascontiguousarray(
            np.asarray(a, dtype=np.float32).reshape((N_CORES,) + shape)
        )

    msg = shard(message, (npc, DEG * F))
    feat = shard(features, (npc, F))
    glob = shard(global_features, (npc, F))
    w1 = np.ascontiguousarray(np.asarray(W1, np.float32))
    w2 = np.ascontiguousarray(np.asarray(W2, np.float32))
    w3 = np.ascontiguousarray(np.asarray(W3, np.float32))
    bb1 = np.ascontiguousarray(np.asarray(b1, np.float32))
    bb2 = np.ascontiguousarray(np.asarray(b2, np.float32))
    bb3 = np.ascontiguousarray(np.asarray(b3, np.float32))

    in_maps = [
        {
            "message": msg[i],
            "features": feat[i],
            "global_features": glob[i],
            "W1": w1,
            "b1": bb1,
            "W2": w2,
            "b2": bb2,
            "W3": w3,
            "b3": bb3,
        }
        for i in range(N_CORES)
    ]
    res = run_bass_kernel_spmd(nc, in_maps, list(range(N_CORES))).results
    return np.concatenate([res[i]["out"] for i in range(N_CORES)], axis=0)


# revision 5
# speedup vs baseline: 1.1631x; 1.1631x over previous
"""Trainium2 Bass kernel for nn_NodeNetwork (GNN message passing + MLP + L2 norm).

Data-parallel over nodes: 500000 nodes sharded as 62500/core across 8
NeuronCores; MLP weights replicated. Per core, nodes are processed in 125
macro-tiles of 500 nodes.

Within a macro-tile, node 500*mi + 4*p + g maps to partition p (125 used),
group g in [0,4).  This makes every message DMA read one dense 4 MB DRAM
region as 125 x 32 KB contiguous per-partition chunks — the layout the
HBM DMA path needs to run near line rate (8 KB strided descriptors, as in
the node = g*125+p mapping, measure ~2.3x slower).  Features/global/out
move as 2 KB-per-partition dense blocks.  Output stores go through the
otherwise-idle GPSIMD (SWDGE) queue so the Activation engine's instruction
stream never blocks on them.
"""

import numpy as np

F = 128
DEG = 16
H1 = 256
H2 = 256
OUT = 128
N_CORES = 8

G = 125          # partitions used per macro-tile
NG = 4           # nodes per partition per macro-tile
MACRO = G * NG   # 500 nodes per macro-tile

_NC_CACHE = {}


def build(n_nodes, n_cores=N_CORES, mode="full"):
    import concourse.bacc as bacc
    import concourse.mybir as mybir
    import concourse.tile as tile
    import concourse.masks as masks
    from contextlib import ExitStack

    f32 = mybir.dt.float32
    AX = mybir.AxisListType
    ALU = mybir.AluOpType
    ACTF = mybir.ActivationFunctionType

    assert n_nodes % MACRO == 0
    n_macros = n_nodes // MACRO

    nc = bacc.Bacc(
        "TRN2", target_bir_lowering=False, debug=False, num_devices=n_cores
    )
    msg_d = nc.dram_tensor("message", [n_nodes, DEG * F], f32, kind="ExternalInput").ap()
    feat_d = nc.dram_tensor("features", [n_nodes, F], f32, kind="ExternalInput").ap()
    glob_d = nc.dram_tensor(
        "global_features", [n_nodes, F], f32, kind="ExternalInput"
    ).ap()
    w1_d = nc.dram_tensor("W1", [3 * F, H1], f32, kind="ExternalInput").ap()
    b1_d = nc.dram_tensor("b1", [H1], f32, kind="ExternalInput").ap()
    w2_d = nc.dram_tensor("W2", [H1, H2], f32, kind="ExternalInput").ap()
    b2_d = nc.dram_tensor("b2", [H2], f32, kind="ExternalInput").ap()
    w3_d = nc.dram_tensor("W3", [H2, OUT], f32, kind="ExternalInput").ap()
    b3_d = nc.dram_tensor("b3", [OUT], f32, kind="ExternalInput").ap()
    out_d = nc.dram_tensor("out", [n_nodes, OUT], f32, kind="ExternalOutput").ap()

    with tile.TileContext(nc) as tc, ExitStack() as ctx:
        cpool = ctx.enter_context(tc.tile_pool(name="consts", bufs=1))
        mpool = ctx.enter_context(tc.tile_pool(name="msg", bufs=4))
        ipool = ctx.enter_context(tc.tile_pool(name="inputs", bufs=3))
        xpool = ctx.enter_context(tc.tile_pool(name="xside", bufs=3))
        xtpool = ctx.enter_context(tc.tile_pool(name="xt", bufs=4))
        hpool = ctx.enter_context(tc.tile_pool(name="hid", bufs=2))
        npool = ctx.enter_context(tc.tile_pool(name="norm", bufs=2))
        opool = ctx.enter_context(tc.tile_pool(name="outp", bufs=3))
        ps_xt = ctx.enter_context(tc.tile_pool(name="ps_xt", bufs=2, space="PSUM"))
        ps_mm = ctx.enter_context(tc.tile_pool(name="ps_mm", bufs=3, space="PSUM"))
        ps_out = ctx.enter_context(tc.tile_pool(name="ps_out", bufs=2, space="PSUM"))

        # --- constants ---
        ident = cpool.tile([128, 128], f32, tag="ident")
        masks.make_identity(nc, ident[:])
        w1sb = cpool.tile([128, 3 * H1], f32, tag="w1")  # [p, (k=3, m*128+c=256)]
        nc.sync.dma_start(w1sb[:], w1_d.rearrange("(k p) m -> p k m", p=128))
        w2sb = cpool.tile([128, 2 * H2], f32, tag="w2")
        nc.sync.dma_start(w2sb[:], w2_d.rearrange("(k p) m -> p k m", p=128))
        w3sb = cpool.tile([128, 2 * OUT], f32, tag="w3")
        nc.sync.dma_start(w3sb[:], w3_d.rearrange("(k p) m -> p k m", p=128))
        b1sb = cpool.tile([128, 2], f32, tag="b1")
        nc.sync.dma_start(b1sb[:], b1_d.rearrange("(m p) -> p m", p=128))
        b2sb = cpool.tile([128, 2], f32, tag="b2")
        nc.sync.dma_start(b2sb[:], b2_d.rearrange("(m p) -> p m", p=128))
        b3sb = cpool.tile([128, 1], f32, tag="b3")
        nc.sync.dma_start(b3sb[:], b3_d.rearrange("(m p) -> p m", p=128))

        if mode == "noin":
            z = opool.tile([128, NG * F], f32, tag="outsb")
            nc.gpsimd.memset(z[:], 0.25)
            for mi in range(n_macros):
                r0 = mi * MACRO
                nc.gpsimd.dma_start(
                    out_d[r0 : r0 + MACRO].rearrange("(p g) f -> p g f", p=G),
                    z[:G].rearrange("p (g f) -> p g f", g=NG),
                )
            n_macros = 0

        for mi in range(n_macros):
            r0 = mi * MACRO
            # --- loads: msg is one dense 4 MB region, 32 KB/partition ---
            msgt = mpool.tile([128, NG * DEG * F], f32, tag="msgt")
            nc.sync.dma_start(
                msgt[:G].rearrange("p (g j) -> p g j", g=NG),
                msg_d[r0 : r0 + MACRO].rearrange("(p g) j -> p g j", p=G),
            )
            featt = ipool.tile([128, NG * F], f32, tag="featt")
            nc.sync.dma_start(
                featt[:G].rearrange("p (g f) -> p g f", g=NG),
                feat_d[r0 : r0 + MACRO].rearrange("(p g) f -> p g f", p=G),
            )
            globt = ipool.tile([128, NG * F], f32, tag="globt")
            nc.sync.dma_start(
                globt[:G].rearrange("p (g f) -> p g f", g=NG),
                glob_d[r0 : r0 + MACRO].rearrange("(p g) f -> p g f", p=G),
            )

            if mode != "full":
                xagg = xpool.tile([128, NG * F], f32, tag="xagg")
                if mode == "dma_agg":
                    for g in range(NG):
                        nc.vector.tensor_reduce(
                            xagg[:G, g * F : (g + 1) * F],
                            msgt[:G, g * DEG * F : (g + 1) * DEG * F].rearrange(
                                "p (d f) -> p f d", f=F
                            ),
                            axis=AX.X,
                            op=ALU.add,
                        )
                else:
                    nc.vector.tensor_copy(xagg[:G], msgt[:G, : NG * F])
                outsb = opool.tile([128, NG * F], f32, tag="outsb")
                nc.vector.tensor_add(outsb[:G], xagg[:G], featt[:G])
                nc.gpsimd.dma_start(
                    out_d[r0 : r0 + MACRO].rearrange("(p g) f -> p g f", p=G),
                    outsb[:G].rearrange("p (g f) -> p g f", g=NG),
                )
                continue

            # --- mailbox sum over DEG (node-major) ---
            xagg = xpool.tile([128, NG * F], f32, tag="xagg")
            for g in range(NG):
                nc.vector.tensor_reduce(
                    xagg[:G, g * F : (g + 1) * F],
                    msgt[:G, g * DEG * F : (g + 1) * DEG * F].rearrange(
                        "p (d f) -> p f d", f=F
                    ),
                    axis=AX.X,
                    op=ALU.add,
                )

            # --- transpose x pieces to feature-major [128, 500] ---
            xts = []
            for src in (xagg, featt, globt):
                pxt = ps_xt.tile([128, 512], f32, tag="pxt")
                for g in range(NG):
                    nc.tensor.transpose(
                        pxt[:, g * G : (g + 1) * G],
                        src[:G, g * F : (g + 1) * F],
                        ident[:G, :G],
                    )
                xt = xtpool.tile([128, MACRO], f32, tag="xt")
                nc.scalar.copy(xt[:], pxt[:, :MACRO])
                xts.append(xt)

            # --- layer 1: [384 -> 256], relu ---
            h1 = hpool.tile([128, 2 * MACRO], f32, tag="h1")
            for m in range(2):
                pmm = ps_mm.tile([128, MACRO], f32, tag="pmm")
                for k in range(3):
                    nc.tensor.matmul(
                        pmm[:],
                        w1sb[:, k * H1 + m * 128 : k * H1 + (m + 1) * 128],
                        xts[k][:],
                        start=(k == 0),
                        stop=(k == 2),
                    )
                nc.scalar.activation(
                    h1[:, m * MACRO : (m + 1) * MACRO],
                    pmm[:],
                    ACTF.Relu,
                    bias=b1sb[:, m : m + 1],
                )

            # --- layer 2: [256 -> 256], relu ---
            h2 = hpool.tile([128, 2 * MACRO], f32, tag="h2")
            for m in range(2):
                pmm = ps_mm.tile([128, MACRO], f32, tag="pmm")
                for k in range(2):
                    nc.tensor.matmul(
                        pmm[:],
                        w2sb[:, k * H2 + m * 128 : k * H2 + (m + 1) * 128],
                        h1[:, k * MACRO : (k + 1) * MACRO],
                        start=(k == 0),
                        stop=(k == 1),
                    )
                nc.scalar.activation(
                    h2[:, m * MACRO : (m + 1) * MACRO],
                    pmm[:],
                    ACTF.Relu,
                    bias=b2sb[:, m : m + 1],
                )

            # --- layer 3: [256 -> 128], + b3 ---
            pmm = ps_mm.tile([128, MACRO], f32, tag="pmm")
            for k in range(2):
                nc.tensor.matmul(
                    pmm[:],
                    w3sb[:, k * OUT : (k + 1) * OUT],
                    h2[:, k * MACRO : (k + 1) * MACRO],
                    start=(k == 0),
                    stop=(k == 1),
                )
            o3 = hpool.tile([128, MACRO], f32, tag="o3")
            nc.scalar.activation(o3[:], pmm[:], ACTF.Identity, bias=b3sb[:, 0:1])

            # --- transpose back to node-major ---
            pout = ps_out.tile([128, NG * F], f32, tag="pout")
            for g in range(NG):
                nc.tensor.transpose(
                    pout[:G, g * F : (g + 1) * F],
                    o3[:, g * G : (g + 1) * G],
                    ident[:, :],
                )

            # --- row L2 norm ---
            sq = npool.tile([128, NG * F], f32, tag="sq")
            nsq = npool.tile([128, NG], f32, tag="nsq")
            for g in range(NG):
                nc.scalar.activation(
                    sq[:G, g * F : (g + 1) * F],
                    pout[:G, g * F : (g + 1) * F],
                    ACTF.Square,
                    accum_out=nsq[:G, g : g + 1],
                )
            nv = npool.tile([128, NG], f32, tag="nv")
            nc.scalar.activation(nv[:G], nsq[:G], ACTF.Sqrt)
            nve = npool.tile([128, NG], f32, tag="nve")
            nc.vector.tensor_scalar_add(nve[:G], nv[:G], 1e-8)
            ri = npool.tile([128, NG], f32, tag="ri")
            nc.vector.reciprocal(ri[:G], nve[:G])

            outsb = opool.tile([128, NG * F], f32, tag="outsb")
            for g in range(NG):
                nc.vector.tensor_scalar_mul(
                    outsb[:G, g * F : (g + 1) * F],
                    pout[:G, g * F : (g + 1) * F],
                    ri[:G, g : g + 1],
                )

            # --- store (GPSIMD/SWDGE queue; 2 KB dense per partition) ---
            nc.gpsimd.dma_start(
                out_d[r0 : r0 + MACRO].rearrange("(p g) f -> p g f", p=G),
                outsb[:G].rearrange("p (g f) -> p g f", g=NG),
            )

    nc.compile()
    return nc


def _get_nc(n_nodes, n_cores):
    key = (n_nodes, n_cores)
    if key not in _NC_CACHE:
        _NC_CACHE[key] = build(n_nodes, n_cores)
    return _NC_CACHE[key]


def kernel(message, features, global_features, W1, b1, W2, b2, W3, b3):
    from concourse.bass_utils import run_bass_kernel_spmd

    n = message.shape[0]
    assert n % N_CORES == 0
    npc = n // N_CORES

    nc = _get_nc(npc, N_CORES)

    def shard(a, shape):
        return np.ascontiguousarray(
            np.asarray(a, dtype=np.float32).reshape((N_CORES,) + shape)
        )

    msg = shard(message, (npc, DEG * F))
    feat = shard(features, (npc, F))
    glob = shard(global_features, (npc, F))
    w1 = np.ascontiguousarray(np.asarray(W1, np.float32))
    w2 = np.ascontiguousarray(np.asarray(W2, np.float32))
    w3 = np.ascontiguousarray(np.asarray(W3, np.float32))
    bb1 = np.ascontiguousarray(np.asarray(b1, np.float32))
    bb2 = np.ascontiguousarray(np.asarray(b2, np.float32))
    bb3 = np.ascontiguousarray(np.asarray(b3, np.float32))

    in_maps = [
        {
            "message": msg[i],
            "features": feat[i],
            "global_features": glob[i],
            "W1": w1,
            "b1": bb1,
            "W2": w2,
            "b2": bb2,
            "W3": w3,
            "b3": bb3,
        }
        for i in range(N_CORES)
    ]
    res = run_bass_kernel_spmd(nc, in_maps, list(range(N_CORES))).results
    return np.concatenate([res[i]["out"] for i in range(N_CORES)], axis=0)
